# revision 1
# baseline (speedup 1.0000x reference)
"""Trainium2 Bass kernel for the DimeNet-style directed-message block.

Reference computation (W = n_angles, E = n_edges, D = 128, A = 49, J = 8):
    m_kj     = m_ji[kj_idx]                          # [W, D]
    transf_m = silu(m_kj @ W_nbr + b_nbr)            # [W, D]
    transf_e = e_rbf[kj_idx] @ W_e                   # [W, D]
    m_and_e  = transf_m * transf_e                   # [W, D]
    transf_a = a_sbf @ W_a                           # [W, J]
    out[w,i] = sum_{j,l} transf_a[w,j] m_and_e[w,l] final_w[i,j,l]
    final    = segment_sum(out, kj_idx, E)           # [E, D]

Algebraic refactor: every per-angle factor except transf_a depends on the
angle only through kj_idx, so the segment sum commutes through the bilinear
form:
    me       = silu(m_ji @ W_nbr + b) * (e_rbf @ W_e)        # [E, D]
    S        = segment_sum(a_sbf @ W_a, kj_idx, E)           # [E, J]
    final[e] = sum_j S[e,j] * (me[e] @ final_w[:,j,:].T)     # [E, D]

S is computed without any scatter for the common case: the host bins each
edge's angles into rank slots (rank r = r-th angle of its edge) and lays out
a_sbf^T so that rank pass r streams through the PE aligned by edge; PSUM
accumulation over the rank passes IS the segment sum. Edges with more than
R0 angles spill into compacted overflow levels whose partial sums are
scatter-added (dma_scatter_add) with *unique* indices per call — duplicate
indices inside one scatter call race on the CCE read-modify-write path and
lose updates (measured), unique ones are exact.

Sharding: edges are contiguous, 25000 per core; angles are binned by owner
core (kj // 25000) so scatter indices fit int16 and no collective is needed.
"""

import numpy as np

import concourse.bass as bass
import concourse.mybir as mybir
import concourse.tile as tile
from concourse import bacc, bass_utils, library_config

F32 = mybir.dt.float32
I16 = mybir.dt.int16
AF = mybir.ActivationFunctionType
OP = mybir.AluOpType

D = 128
A_DIM = 49
N_RBF = 6
N_BIL = 8
N_CORES = 8
TRASH_ROW = 25_000      # scatter target for padding tokens (adds zeros)
MAX_SCATTER = 4096      # tokens per dma_scatter_add call (ring limit)


class Cfg:
    """levels: tuple of (cap_i, R_i); level 0 cap must equal e_pad."""

    def __init__(self, e_valid, e_pad, levels):
        self.e_valid = e_valid
        self.e_pad = e_pad
        self.levels = tuple(levels)
        assert e_pad % 2048 == 0
        assert levels[0][0] == e_pad
        for cap, r in levels:
            assert cap % 512 == 0 and r % 2 == 0
        self.n_chunks_b = e_pad // 1024
        # packed aT stream: one 512-col block per (level, edge-chunk, rank
        # pair); host pads the array to a 2048-col multiple.
        self.n_blocks = sum((cap // 512) * (r // 2) for cap, r in levels)
        self.at_cols = ((self.n_blocks * 512 + 2047) // 2048) * 2048

    def key(self):
        return (self.e_valid, self.e_pad, self.levels)


def build_nc(cfg: Cfg, phases=(1, 2)):
    nc = bacc.Bacc(None)
    EP = cfg.e_pad

    aT = nc.dram_tensor("a_t", [64 + A_DIM, cfg.at_cols], F32,
                        kind="ExternalInput")
    mjiT = nc.dram_tensor("mji_t", [D, EP], F32, kind="ExternalInput")
    erbf = nc.dram_tensor("erbf_t", [N_RBF, EP], F32, kind="ExternalInput")
    wnbr = nc.dram_tensor("w_nbr", [D, D], F32, kind="ExternalInput")
    bnbr = nc.dram_tensor("b_nbr", [D, 1], F32, kind="ExternalInput")
    wes = nc.dram_tensor("w_e", [N_RBF, D], F32, kind="ExternalInput")
    wa2 = nc.dram_tensor("w_a2", [64 + A_DIM, N_BIL], F32,
                         kind="ExternalInput")
    i8d = nc.dram_tensor("i8", [N_BIL, N_BIL], F32, kind="ExternalInput")
    t2 = nc.dram_tensor("t2", [D, N_BIL * D], F32, kind="ExternalInput")
    idxd = {}
    for li, (cap, _r) in enumerate(cfg.levels):
        if li == 0:
            continue
        idxd[li] = nc.dram_tensor(f"idx_l{li}", [128, cap // 16], I16,
                                  kind="ExternalInput")
    outd = nc.dram_tensor("out", [EP, D], F32, kind="ExternalOutput")
    sovf = nc.dram_tensor("s_ovf", [EP, 64], F32)   # internal, 256B rows

    n_groups0 = EP // 128

    with tile.TileContext(nc) as tc:
        nc.gpsimd.load_library(library_config.mlp)
        with tc.tile_pool(name="const", bufs=1) as cp:
            wa_sb = cp.tile([64 + A_DIM, N_BIL], F32)
            nc.sync.dma_start(out=wa_sb[:], in_=wa2[:])
            i8_sb = cp.tile([N_BIL, N_BIL], F32)
            nc.sync.dma_start(out=i8_sb[:], in_=i8d[:])
            s_sbuf = cp.tile([128, n_groups0 * N_BIL], F32)

            # ---- zero the overflow accumulator ----
            s_flat = sovf.ap().rearrange("(p x) c -> p (x c)", p=128)
            zcols = s_flat.shape[1]
            with tc.tile_pool(name="zero", bufs=1) as zp:
                zt = zp.tile([128, zcols // 4], F32)
                nc.vector.memset(zt[:], 0.0)
                for q in range(4):
                    nc.sync.dma_start(
                        out=s_flat[:, q * (zcols // 4):(q + 1) * (zcols // 4)],
                        in_=zt[:])

            # ============ phase A: S via rank-pass PSUM accumulation =======
            if 1 not in phases:
                nc.vector.memset(s_sbuf[:], 0.0)
            if 1 in phases:
              with tc.tile_pool(name="pa", bufs=3) as pa, \
                 tc.tile_pool(name="stp", bufs=2) as stp, \
                 tc.tile_pool(name="stage", bufs=1) as stage, \
                 tc.tile_pool(name="pss", bufs=2, space="PSUM") as pss, \
                 tc.tile_pool(name="pst", bufs=2, space="PSUM") as pst:
                at_tiles = {}

                def at_block(b):
                    ck = b // 4
                    if ck not in at_tiles:
                        t = pa.tile([64 + A_DIM, 2048], F32, tag="at")
                        nc.sync.dma_start(
                            out=t[:], in_=aT[:, ck * 2048:(ck + 1) * 2048])
                        at_tiles.clear()
                        at_tiles[ck] = t
                    off = (b % 4) * 512
                    return at_tiles[ck][:, off:off + 512]

                stages = {}
                idx_sb = {}
                blk = 0
                for li, (cap, R) in enumerate(cfg.levels):
                    n_groups = cap // 128
                    if li > 0:
                        stages[li] = stage.tile([128, n_groups * N_BIL], F32,
                                                tag=f"stage{li}",
                                                name=f"stage{li}")
                        idx_sb[li] = stage.tile([128, cap // 16], I16,
                                                tag=f"idx{li}",
                                                name=f"idx{li}")
                        nc.sync.dma_start(out=idx_sb[li][:], in_=idxd[li][:])
                    pt = None
                    for c in range(cap // 512):
                        # A psum accumulation group must keep one lhsT
                        # partition base (base switches mid-group wedge the
                        # PE): even ranks (base 0) and odd ranks (base 64)
                        # accumulate separately, merged on DVE.
                        abs_ = []
                        for p in range(R // 2):
                            abs_.append(at_block(blk))
                            blk += 1
                        ps_e = pss.tile([N_BIL, 512], F32, tag="se")
                        for p in range(R // 2):
                            nc.tensor.matmul(
                                ps_e[:], wa_sb[0:A_DIM, :],
                                abs_[p][0:A_DIM, :],
                                start=(p == 0), stop=(p == R // 2 - 1))
                        ps_o = pss.tile([N_BIL, 512], F32, tag="so")
                        for p in range(R // 2):
                            nc.tensor.matmul(
                                ps_o[:], wa_sb[64:64 + A_DIM, :],
                                abs_[p][64:64 + A_DIM, :],
                                start=(p == 0), stop=(p == R // 2 - 1))
                        st = stp.tile([N_BIL, 512], F32, tag="st")
                        nc.vector.tensor_copy(out=st[:], in_=ps_e[:])
                        nc.vector.tensor_add(st[:], st[:], ps_o[:])
                        # transpose [8,128] pieces -> [128,8] psum slots
                        for q in range(4):
                            gl = c * 4 + q
                            slot = gl % 64
                            if slot == 0:
                                pt = pst.tile([128, 512], F32, tag="tp")
                            nc.tensor.matmul(
                                pt[:, slot * 8:(slot + 1) * 8],
                                st[:, q * 128:(q + 1) * 128],
                                i8_sb[:], start=True, stop=True)
                            if slot == 63 or gl == n_groups - 1:
                                g0 = gl - slot
                                dst = s_sbuf if li == 0 else stages[li]
                                nc.vector.tensor_copy(
                                    out=dst[:, g0 * 8:(gl + 1) * 8],
                                    in_=pt[:, :(slot + 1) * 8])
                    # overflow scatter (unique indices per call)
                    if li > 0:
                        t0 = 0
                        while t0 < cap:
                            n_tok = min(MAX_SCATTER, cap - t0)
                            in_ap = stages[li][:, t0 // 128 * 8:
                                               (t0 + n_tok) // 128 * 8]
                            nc.gpsimd.dma_scatter_add(
                                out_ap=sovf[:, 0:N_BIL],
                                in_ap=in_ap.rearrange("p (c e) -> p c e",
                                                      e=N_BIL),
                                idxs_ap=idx_sb[li][:, t0 // 16:
                                                   (t0 + n_tok) // 16],
                                num_idxs=n_tok,
                                num_idxs_reg=n_tok,
                                elem_size=N_BIL,
                                elem_step=64,
                                queue_num=0)
                            t0 += n_tok

            # ============ phase B: edge transform + S apply ================
            if 2 not in phases:
                with tc.tile_pool(name="dbg", bufs=1) as dbg:
                    dtile = dbg.tile([128, n_groups0 * N_BIL], F32)
                    nc.vector.tensor_copy(out=dtile[:], in_=s_sbuf[:])
                    ov = outd.ap().rearrange("(p x) c -> p (x c)", p=128)
                    nc.sync.dma_start(out=ov[:, 0:n_groups0 * N_BIL],
                                      in_=dtile[:])
            if 2 in phases:
              wn_sb = cp.tile([D, D], F32)
              nc.sync.dma_start(out=wn_sb[:], in_=wnbr[:])
              b_sb = cp.tile([D, 1], F32)
              nc.sync.dma_start(out=b_sb[:], in_=bnbr[:])
              we_sb = cp.tile([N_RBF, D], F32)
              nc.sync.dma_start(out=we_sb[:], in_=wes[:])
              t2_sb = cp.tile([D, N_BIL * D], F32)
              nc.sync.dma_start(out=t2_sb[:], in_=t2[:])

              s_view = sovf.ap().rearrange("(t p) c -> p t c", p=128)

              with tc.tile_pool(name="pb", bufs=2) as pb, \
                   tc.tile_pool(name="sbp", bufs=2) as sbp, \
                   tc.tile_pool(name="accp", bufs=3) as accp, \
                   tc.tile_pool(name="psmm", bufs=2, space="PSUM") as pmm, \
                   tc.tile_pool(name="psy", bufs=2, space="PSUM") as py:
                  for c in range(cfg.n_chunks_b):
                      er_sb = pb.tile([N_RBF, 1024], F32, tag="er")
                      nc.sync.dma_start(out=er_sb[:],
                                        in_=erbf[:, c * 1024:(c + 1) * 1024])
                      te_ps = pmm.tile([128, 1024], F32, tag="mm")
                      for n in range(2):
                          nc.tensor.matmul(
                              te_ps[:, n * 512:(n + 1) * 512],
                              we_sb[:], er_sb[:, n * 512:(n + 1) * 512],
                              start=True, stop=True)
                      mj = pb.tile([128, 1024], F32, tag="mj")
                      nc.sync.dma_start(out=mj[:],
                                        in_=mjiT[:, c * 1024:(c + 1) * 1024])
                      tm_ps = pmm.tile([128, 1024], F32, tag="mm")
                      for n in range(2):
                          nc.tensor.matmul(
                              tm_ps[:, n * 512:(n + 1) * 512],
                              wn_sb[:], mj[:, n * 512:(n + 1) * 512],
                              start=True, stop=True)
                      sg_sb = pb.tile([128, 1024], F32, tag="sg")
                      nc.scalar.activation(sg_sb[:], tm_ps[:], AF.Sigmoid,
                                           bias=b_sb[:, 0:1])
                      tm_sb = pb.tile([128, 1024], F32, tag="tm")
                      # silu(x+b) = (x+b) * sigmoid(x+b)
                      nc.vector.scalar_tensor_tensor(
                          out=tm_sb[:], in0=tm_ps[:], scalar=b_sb[:, 0:1],
                          in1=sg_sb[:], op0=OP.add, op1=OP.mult)
                      me_sb = pb.tile([128, 1024], F32, tag="me")
                      nc.vector.tensor_mul(me_sb[:], tm_sb[:], te_ps[:])

                      so_sb = sbp.tile([128, 8 * 64], F32, tag="so")
                      nc.sync.dma_start(out=so_sb[:],
                                        in_=s_view[:, c * 8:(c + 1) * 8, :])
                      s_tot = sbp.tile([128, 64], F32, tag="stot")
                      nc.vector.tensor_add(
                          s_tot[:].rearrange("p (t j) -> p t j", j=8),
                          s_sbuf[:, c * 64:(c + 1) * 64]
                          .rearrange("p (t j) -> p t j", j=8),
                          so_sb[:].rearrange("p (t j) -> p t j", j=64)
                          [:, :, 0:8])

                      for tt in range(8):
                          y = py.tile([128, N_BIL * D], F32, tag="y")
                          lhsT = me_sb[:, tt * 128:(tt + 1) * 128]
                          nc.tensor.matmul(y[:, 0:512], lhsT, t2_sb[:, 0:512],
                                           start=True, stop=True)
                          nc.tensor.matmul(y[:, 512:1024], lhsT,
                                           t2_sb[:, 512:1024],
                                           start=True, stop=True)
                          acc = accp.tile([128, D], F32, tag="acc")
                          nc.vector.tensor_scalar_mul(
                              acc[:], y[:, 0:D], s_tot[:, tt * 8:tt * 8 + 1])
                          for j in range(1, N_BIL):
                              nc.vector.scalar_tensor_tensor(
                                  out=acc[:],
                                  in0=y[:, j * D:(j + 1) * D],
                                  scalar=s_tot[:, tt * 8 + j:tt * 8 + j + 1],
                                  in1=acc[:],
                                  op0=OP.mult, op1=OP.add)
                          e0 = (c * 8 + tt) * 128
                          nc.sync.dma_start(out=outd[e0:e0 + 128, :], in_=acc[:])
    nc.finalize()
    return nc


# ----------------------------------------------------------------------------
# host-side sharding / unsharding
# ----------------------------------------------------------------------------

def make_cfg(kj, n_edges, ev=25_000, ep=26_624):
    n_cores = (n_edges + ev - 1) // ev
    owner = np.minimum(kj // ev, n_cores - 1)
    caps = []  # per level >=1: max count over cores
    max_rank = 0
    for c in range(n_cores):
        loc = kj[owner == c] - c * ev
        cnt = np.bincount(loc, minlength=ev)
        max_rank = max(max_rank, int(cnt.max()))
        base = 4
        li = 0
        while (cnt > base).any():
            n = int((cnt > base).sum())
            if li >= len(caps):
                caps.append(n)
            else:
                caps[li] = max(caps[li], n)
            base += 4
            li += 1
    levels = [(ep, 4)]
    for n in caps:
        levels.append((max(512, ((n + 511) // 512) * 512), 4))
    return Cfg(ev, ep, levels)


def prep_in_maps(cfg: Cfg, m_ji, nbr_list, angle_list, e_rbf, a_sbf, kj_idx,
                 W_nbr, b_nbr, W_e, W_a, final_w):
    del nbr_list, angle_list
    m_ji = np.asarray(m_ji, np.float32)
    e_rbf = np.asarray(e_rbf, np.float32)
    a_sbf = np.asarray(a_sbf, np.float32)
    kj = np.asarray(kj_idx).astype(np.int64)
    W_nbr = np.asarray(W_nbr, np.float32)
    b_nbr = np.asarray(b_nbr, np.float32)
    W_e = np.asarray(W_e, np.float32)
    W_a = np.asarray(W_a, np.float32)
    final_w = np.asarray(final_w, np.float32)

    n_edges = m_ji.shape[0]
    ev = cfg.e_valid
    ep = cfg.e_pad
    n_cores = (n_edges + ev - 1) // ev
    owner = np.minimum(kj // ev, n_cores - 1)

    wa2 = np.zeros((64 + A_DIM, N_BIL), np.float32)
    wa2[0:A_DIM] = W_a
    wa2[64:64 + A_DIM] = W_a
    t2 = np.ascontiguousarray(final_w.transpose(2, 1, 0).reshape(D, N_BIL * D))
    bn = np.ascontiguousarray(b_nbr.reshape(D, 1))
    i8 = np.eye(N_BIL, dtype=np.float32)

    in_maps = []
    for c in range(n_cores):
        sel = np.nonzero(owner == c)[0]
        loc = kj[sel] - c * ev
        order = np.argsort(loc, kind="stable")
        loc = loc[order]
        rows = sel[order]                       # a_sbf row per sorted token
        cnt = np.bincount(loc, minlength=ep)
        starts = np.concatenate([[0], np.cumsum(cnt)])

        # pack the rank-pass stream
        at = np.zeros((64 + A_DIM, cfg.at_cols), np.float32)
        col = 0
        base = 0
        idx_maps = {}
        for li, (cap, R) in enumerate(cfg.levels):
            if li == 0:
                elist = np.arange(ep)
            else:
                elist = np.nonzero(cnt > base)[0]
                assert len(elist) <= cap, (li, len(elist), cap)
                el_pad = np.full(cap, cfg.e_valid, np.int64)
                el_pad[:len(elist)] = elist
                w16 = el_pad.astype(np.int16).reshape(-1, 16).T
                idx_maps[f"idx_l{li}"] = np.ascontiguousarray(
                    np.tile(w16, (8, 1)))
            # A_r [cap, 49] per rank
            a_rs = []
            for r in range(R):
                a_r = np.zeros((cap, A_DIM), np.float32)
                has = np.nonzero(cnt[elist] > base + r)[0]  # pos within elist
                tok = starts[elist[has]] + base + r
                a_r[has] = a_sbf[rows[tok]]
                a_rs.append(a_r)
            for cc in range(cap // 512):
                for p in range(R // 2):
                    at[0:A_DIM, col:col + 512] = \
                        a_rs[2 * p][cc * 512:(cc + 1) * 512].T
                    at[64:64 + A_DIM, col:col + 512] = \
                        a_rs[2 * p + 1][cc * 512:(cc + 1) * 512].T
                    col += 512
            base += R
        assert int(cnt.max()) <= base, "levels do not cover max multiplicity"

        e0, e1 = c * ev, min((c + 1) * ev, n_edges)
        mjiT = np.zeros((D, ep), np.float32)
        mjiT[:, :e1 - e0] = m_ji[e0:e1].T
        erbfT = np.zeros((N_RBF, ep), np.float32)
        erbfT[:, :e1 - e0] = e_rbf[e0:e1].T

        im = {
            "a_t": at, "mji_t": np.ascontiguousarray(mjiT),
            "erbf_t": erbfT, "w_nbr": W_nbr, "b_nbr": bn,
            "w_e": W_e, "w_a2": wa2, "i8": i8, "t2": t2,
        }
        im.update(idx_maps)
        in_maps.append(im)
    return in_maps


def gather_output(cfg: Cfg, results, n_edges):
    outs = []
    ev = cfg.e_valid
    for c, r in enumerate(results):
        e0, e1 = c * ev, min((c + 1) * ev, n_edges)
        outs.append(np.asarray(r["out"])[:e1 - e0])
    return np.ascontiguousarray(np.concatenate(outs, axis=0))


_NC_CACHE = {}


def run_on_hw(inputs, cfg=None, trace=False, trace_cores=None):
    kj = np.asarray(inputs["kj_idx"]).astype(np.int64)
    if cfg is None:
        cfg = make_cfg(kj, inputs["m_ji"].shape[0])
    key = cfg.key()
    if key not in _NC_CACHE:
        _NC_CACHE[key] = build_nc(cfg)
    nc = _NC_CACHE[key]
    in_maps = prep_in_maps(cfg, **inputs)
    res = bass_utils.run_bass_kernel_spmd(
        nc, in_maps, core_ids=list(range(len(in_maps))),
        trace=trace, trace_cores=trace_cores)
    out = gather_output(cfg, res.results, inputs["m_ji"].shape[0])
    return out, res


def kernel(**inputs) -> np.ndarray:
    out, _ = run_on_hw(inputs)
    return out



# revision 6
# speedup vs baseline: 4.8936x; 4.8936x over previous
"""Trainium2 Bass kernel for the DimeNet-style directed-message block.

Reference computation (W = n_angles, E = n_edges, D = 128, A = 49, J = 8):
    m_kj     = m_ji[kj_idx]                          # [W, D]
    transf_m = silu(m_kj @ W_nbr + b_nbr)            # [W, D]
    transf_e = e_rbf[kj_idx] @ W_e                   # [W, D]
    m_and_e  = transf_m * transf_e                   # [W, D]
    transf_a = a_sbf @ W_a                           # [W, J]
    out[w,i] = sum_{j,l} transf_a[w,j] m_and_e[w,l] final_w[i,j,l]
    final    = segment_sum(out, kj_idx, E)           # [E, D]

Algebraic refactor: every per-angle factor except transf_a depends on the
angle only through kj_idx, so the segment sum commutes through the bilinear
form:
    me       = silu(m_ji @ W_nbr + b) * (e_rbf @ W_e)        # [E, D]
    S        = segment_sum(a_sbf @ W_a, kj_idx, E)           # [E, J]
    final[e] = sum_j S[e,j] * (me[e] @ final_w[:,j,:].T)     # [E, D]

S is computed entirely in PSUM with no scatter: edges are SORTED BY ANGLE
MULTIPLICITY within each core, so each 512-edge block has a uniform-ish rank
depth P_b = ceil(max_count/2).  The host lays out a_sbf^T so that rank-pair
pass p of block b streams through the PE aligned by edge slot (rank 2p in
partitions 0-48, rank 2p+1 in partitions 49-97, both multiplied by a
duplicated W_a [98, 8] in one K=98 matmul); PSUM accumulation over the P_b
passes IS the segment sum.  Everything streams in bf16 (fp32 accumulate).

Sharding: edges contiguous, 25000 per core; angles binned by owner core.
All cores run one program built for the per-block envelope max(P_b) over
cores; narrower cores get zero-padded stream columns.
"""

import numpy as np
import ml_dtypes

import concourse.bass as bass
import concourse.mybir as mybir
import concourse.tile as tile
from concourse import bacc, bass_utils, library_config

F32 = mybir.dt.float32
BF16 = mybir.dt.bfloat16
AF = mybir.ActivationFunctionType
OP = mybir.AluOpType
BF = ml_dtypes.bfloat16

D = 128
A_DIM = 49
KA = 2 * A_DIM          # 98: even rank rows 0-48, odd rank rows 49-97
N_RBF = 6
N_BIL = 8
N_CORES = 8
EV = 25_000
EP = 25_088             # 49 * 512
NB = EP // 512          # 49 blocks of 512 edge slots


class Cfg:
    def __init__(self, pb):
        self.pb = tuple(int(x) for x in pb)   # rank-pair passes per block
        assert len(self.pb) == NB
        self.tot_pass = sum(self.pb)
        self.at_cols = self.tot_pass * 512

    def key(self):
        return self.pb


def build_nc(cfg: Cfg):
    nc = bacc.Bacc(None)

    aT = nc.dram_tensor("a_t", [KA, max(cfg.at_cols, 512)], BF16,
                        kind="ExternalInput")
    mjiT = nc.dram_tensor("mji_t", [D, EP], BF16, kind="ExternalInput")
    erbf = nc.dram_tensor("erbf_t", [N_RBF, EP], BF16, kind="ExternalInput")
    wnbr = nc.dram_tensor("w_nbr", [D, D], BF16, kind="ExternalInput")
    bnbr = nc.dram_tensor("b_nbr", [D, 1], F32, kind="ExternalInput")
    wes = nc.dram_tensor("w_e", [N_RBF, D], BF16, kind="ExternalInput")
    wa2 = nc.dram_tensor("w_a2", [KA, N_BIL], BF16, kind="ExternalInput")
    i8d = nc.dram_tensor("i8", [N_BIL, N_BIL], BF16, kind="ExternalInput")
    t2 = nc.dram_tensor("t2", [D, N_BIL * D], BF16, kind="ExternalInput")
    outd = nc.dram_tensor("out", [EP, D], BF16, kind="ExternalOutput")

    p_max = max(max(cfg.pb), 1)

    with tile.TileContext(nc) as tc:
        with tc.tile_pool(name="const", bufs=1) as cp:
            wa_sb = cp.tile([KA, N_BIL], BF16)
            nc.sync.dma_start(out=wa_sb[:], in_=wa2[:])
            i8_sb = cp.tile([N_BIL, N_BIL], BF16)
            nc.sync.dma_start(out=i8_sb[:], in_=i8d[:])
            wn_sb = cp.tile([D, D], BF16)
            nc.sync.dma_start(out=wn_sb[:], in_=wnbr[:])
            b_sb = cp.tile([D, 1], F32)
            nc.sync.dma_start(out=b_sb[:], in_=bnbr[:])
            we_sb = cp.tile([N_RBF, D], BF16)
            nc.sync.dma_start(out=we_sb[:], in_=wes[:])
            t2_sb = cp.tile([D, N_BIL * D], BF16)
            nc.sync.dma_start(out=t2_sb[:], in_=t2[:])
            er_all = cp.tile([N_RBF, EP], BF16)
            nc.sync.dma_start(out=er_all[:], in_=erbf[:])
            # S in slot-edge-partition layout, bf16: [128, (block, tt, j)]
            s_col = cp.tile([D, NB * 32], BF16)
            nc.vector.memset(s_col[:], 0.0)

            with tc.tile_pool(name="pa", bufs=3) as pa, \
                 tc.tile_pool(name="pb", bufs=3) as pbp, \
                 tc.tile_pool(name="pme", bufs=2) as pme, \
                 tc.tile_pool(name="pz", bufs=2) as pz, \
                 tc.tile_pool(name="pacc", bufs=2) as pacc, \
                 tc.tile_pool(name="pss", bufs=1, space="PSUM") as pss, \
                 tc.tile_pool(name="pst", bufs=1, space="PSUM") as pst, \
                 tc.tile_pool(name="pmm", bufs=1, space="PSUM") as pmm, \
                 tc.tile_pool(name="py", bufs=1, space="PSUM") as py:
                col0 = 0
                for b in range(NB):
                    pb_b = cfg.pb[b]
                    # ---------- phase A: S for this block's 512 slots -------
                    if pb_b > 0:
                        at = pa.tile([KA, p_max * 512], BF16, tag="at")
                        nc.sync.dma_start(
                            out=at[:, 0:pb_b * 512],
                            in_=aT[:, col0:col0 + pb_b * 512])
                        col0 += pb_b * 512
                        ps = pss.tile([N_BIL, 512], F32, tag="ps")
                        for p in range(pb_b):
                            nc.tensor.matmul(
                                ps[:], wa_sb[:], at[:, p * 512:(p + 1) * 512],
                                start=(p == 0), stop=(p == pb_b - 1))
                        st = pa.tile([N_BIL, 512], BF16, tag="st")
                        nc.scalar.copy(out=st[:], in_=ps[:])
                        pt = pst.tile([D, 32], F32, tag="pt")
                        for q in range(4):
                            nc.tensor.matmul(
                                pt[:, q * 8:(q + 1) * 8],
                                st[:, q * 128:(q + 1) * 128],
                                i8_sb[:], start=True, stop=True)
                        nc.scalar.copy(out=s_col[:, b * 32:(b + 1) * 32],
                                       in_=pt[:])
                    # ---------- phase B: edges of this block ----------------
                    mj = pbp.tile([D, 512], BF16, tag="mj")
                    nc.sync.dma_start(out=mj[:],
                                      in_=mjiT[:, b * 512:(b + 1) * 512])
                    mm = pmm.tile([D, 1024], F32, tag="mm")
                    nc.tensor.matmul(mm[:, 0:512], wn_sb[:], mj[:],
                                     start=True, stop=True)
                    nc.tensor.matmul(mm[:, 512:1024], we_sb[:],
                                     er_all[:, b * 512:(b + 1) * 512],
                                     start=True, stop=True)
                    silu = pme.tile([D, 512], BF16, tag="silu")
                    nc.scalar.activation(silu[:], mm[:, 0:512], AF.Silu,
                                         bias=b_sb[:, 0:1])
                    te = pme.tile([D, 512], BF16, tag="te")
                    nc.scalar.copy(out=te[:], in_=mm[:, 512:1024])
                    me = pme.tile([D, 512], BF16, tag="me")
                    nc.vector.tensor_mul(me[:], silu[:], te[:])

                    acc = pacc.tile([D, 512], BF16, tag="acc")
                    for g in range(2):           # two groups of 2 edge-tiles
                        y = py.tile([D, 2048], F32, tag="y")
                        for t2i in range(2):
                            lhsT = me[:, (g * 2 + t2i) * 128:
                                      (g * 2 + t2i + 1) * 128]
                            for h in range(2):
                                nc.tensor.matmul(
                                    y[:, t2i * 1024 + h * 512:
                                      t2i * 1024 + (h + 1) * 512],
                                    lhsT, t2_sb[:, h * 512:(h + 1) * 512],
                                    start=True, stop=True)
                        # z[p, (j,t,i)] = y[p, (t,j,i)] * S[p_edge(t), j]
                        z = pz.tile([D, 2048], BF16, tag="z")
                        s_b = s_col[:, b * 32 + g * 16:b * 32 + g * 16 + 16] \
                            .rearrange("p (t j) -> p t j", t=2) \
                            .unsqueeze(3).to_broadcast([D, 2, N_BIL, D])
                        nc.vector.tensor_tensor(
                            out=z[:].rearrange("p (j t i) -> p t j i", j=N_BIL,
                                               t=2),
                            in0=y[:].rearrange("p (t j i) -> p t j i",
                                               j=N_BIL, t=2),
                            in1=s_b, op=OP.mult)
                        # dense bf16 2x add tree over j
                        nc.vector.tensor_add(z[:, 0:1024], z[:, 0:1024],
                                             z[:, 1024:2048])
                        nc.vector.tensor_add(z[:, 0:512], z[:, 0:512],
                                             z[:, 512:1024])
                        nc.vector.tensor_add(acc[:, g * 256:(g + 1) * 256],
                                             z[:, 0:256], z[:, 256:512])
                    ov = outd.ap()[b * 512:(b + 1) * 512, :] \
                        .rearrange("(t p) i -> p t i", p=128)
                    nc.sync.dma_start(
                        out=ov,
                        in_=acc[:].rearrange("p (t i) -> p t i", t=4))
    nc.finalize()
    return nc


# ----------------------------------------------------------------------------
# host-side sharding / unsharding
# ----------------------------------------------------------------------------

def _core_layout(kj):
    """Per-core: cnt, slot order (sorted by multiplicity desc), token rows."""
    owner = np.minimum(kj // EV, N_CORES - 1)
    layouts = []
    for c in range(N_CORES):
        sel = np.nonzero(owner == c)[0]
        loc = kj[sel] - c * EV
        cnt = np.bincount(loc, minlength=EP).astype(np.int64)
        order = np.argsort(-cnt, kind="stable")      # slot -> local edge
        srt = np.argsort(loc, kind="stable")
        rows = sel[srt]                              # token idx -> a_sbf row
        starts = np.concatenate([[0], np.cumsum(cnt)])
        layouts.append((cnt, order, rows, starts))
    return layouts


def make_cfg(layouts):
    pb = np.zeros(NB, np.int64)
    for cnt, order, _rows, _starts in layouts:
        cs = cnt[order]
        for b in range(NB):
            mx = int(cs[b * 512:(b + 1) * 512].max())
            pb[b] = max(pb[b], (mx + 1) // 2)
    return Cfg(pb.tolist())


def prep_in_maps(cfg: Cfg, layouts, m_ji, nbr_list, angle_list, e_rbf, a_sbf,
                 kj_idx, W_nbr, b_nbr, W_e, W_a, final_w):
    del nbr_list, angle_list, kj_idx
    m_ji = np.asarray(m_ji, np.float32)
    e_rbf = np.asarray(e_rbf, np.float32)
    a_sbf = np.asarray(a_sbf, np.float32).astype(BF)
    W_nbr = np.asarray(W_nbr, np.float32)
    b_nbr = np.asarray(b_nbr, np.float32)
    W_e = np.asarray(W_e, np.float32)
    W_a = np.asarray(W_a, np.float32)
    final_w = np.asarray(final_w, np.float32)

    wa2 = np.zeros((KA, N_BIL), np.float32)
    wa2[0:A_DIM] = W_a
    wa2[A_DIM:KA] = W_a
    t2 = np.ascontiguousarray(
        final_w.transpose(2, 1, 0).reshape(D, N_BIL * D))
    bn = np.ascontiguousarray(b_nbr.reshape(D, 1))
    i8 = np.eye(N_BIL, dtype=np.float32)

    common = {
        "w_nbr": W_nbr.astype(BF), "b_nbr": bn, "w_e": W_e.astype(BF),
        "w_a2": wa2.astype(BF), "i8": i8.astype(BF), "t2": t2.astype(BF),
    }

    in_maps = []
    for c in range(N_CORES):
        cnt, order, rows, starts = layouts[c]
        at = np.zeros((KA, max(cfg.at_cols, 512)), BF)
        col = 0
        for b in range(NB):
            pb_b = cfg.pb[b]
            if pb_b == 0:
                continue
            sl = order[b * 512:(b + 1) * 512]        # local edge ids
            cs = cnt[sl]
            st = starts[sl]
            for p in range(pb_b):
                for h, r in ((0, 2 * p), (1, 2 * p + 1)):
                    has = np.nonzero(cs > r)[0]
                    if len(has):
                        tok = st[has] + r
                        at[h * A_DIM:(h + 1) * A_DIM,
                           col + has] = a_sbf[rows[tok]].T
                col += 512
        assert col == cfg.at_cols

        e0, e1 = c * EV, min((c + 1) * EV, m_ji.shape[0])
        mjiT = np.zeros((EP, D), np.float32)
        mjiT[:e1 - e0] = m_ji[e0:e1]
        erbfT = np.zeros((EP, N_RBF), np.float32)
        erbfT[:e1 - e0] = e_rbf[e0:e1]
        im = dict(common)
        im["a_t"] = at
        im["mji_t"] = np.ascontiguousarray(mjiT[order].T).astype(BF)
        im["erbf_t"] = np.ascontiguousarray(erbfT[order].T).astype(BF)
        in_maps.append(im)
    return in_maps


def gather_output(layouts, results, n_edges):
    outs = []
    for c, r in enumerate(results):
        _cnt, order, _rows, _starts = layouts[c]
        inv = np.empty(EP, np.int64)
        inv[order] = np.arange(EP)
        e0, e1 = c * EV, min((c + 1) * EV, n_edges)
        res = np.asarray(r["out"]).astype(np.float32)
        outs.append(res[inv[np.arange(e1 - e0)]])
    return np.ascontiguousarray(np.concatenate(outs, axis=0))


_NC_CACHE = {}


def run_on_hw(inputs, trace=False, trace_cores=None):
    kj = np.asarray(inputs["kj_idx"]).astype(np.int64)
    layouts = _core_layout(kj)
    cfg = make_cfg(layouts)
    key = cfg.key()
    if key not in _NC_CACHE:
        _NC_CACHE[key] = build_nc(cfg)
    nc = _NC_CACHE[key]
    in_maps = prep_in_maps(cfg, layouts, **inputs)
    res = bass_utils.run_bass_kernel_spmd(
        nc, in_maps, core_ids=list(range(len(in_maps))),
        trace=trace, trace_cores=trace_cores)
    out = gather_output(layouts, res.results, inputs["m_ji"].shape[0])
    return out, res


def kernel(**inputs) -> np.ndarray:
    out, _ = run_on_hw(inputs)
    return out
